# revision 10
# baseline (speedup 1.0000x reference)
"""Trainium2 Bass kernel for the CG tensor-product iteration (nn_CGIteration).

out[l3] = f1[l3] + concat_paths(einsum('abm,nak,nbk->nmk', C_p, f1[l1], f2[l2])) @ W[l3]

Self-contained: CG coefficients and the device schedule are computed here.
Data-parallel over nodes N: each of the 8 NeuronCores processes N/8 nodes.

Device algorithm (channel-major layout [k'=128 partitions, comp, n]):
  - products  P[m1,m2] = f1[l1,m1] * f2[l2,m2]   (DVE bf16 tensor_tensor,
    several (m1, m2-run) components per instruction via broadcast APs)
  - equal-|c| group sums S_g = sum_i sign_i P_i   (DVE add/sub)
  - PE matmuls psum[l3,m3] += (sign*|c| * W_path).T @ S_g with PSUM
    accumulation; the residual f1[l3] enters as an identity matmul
  - ACT evicts psum -> SBUF f32, DMA to DRAM
Weight variants (coeff * W_path, bf16) are prescaled on the host and DMAed.
"""

import sys
from math import factorial as fact

import numpy as np

if "/opt/trn_rl_repo" not in sys.path:  # harness safety; usually importable
    sys.path.append("/opt/trn_rl_repo")

import ml_dtypes

import concourse.mybir as mybir
import concourse.tile as tile
from concourse import bacc
from concourse.bass_utils import run_bass_kernel_spmd

BF16 = mybir.dt.bfloat16
F32 = mybir.dt.float32

L_MAX = 3
K = 128
N_TOTAL = 8192
N_CORES = 8
N_LOCAL = N_TOTAL // N_CORES

PATHS = [
    (l1, l2, l3)
    for l1 in range(L_MAX + 1)
    for l2 in range(L_MAX + 1)
    for l3 in range(abs(l1 - l2), min(l1 + l2, L_MAX) + 1)
]

PATH_BLOCK = {}
_counts = [0, 0, 0, 0]
for _p in PATHS:
    PATH_BLOCK[_p] = _counts[_p[2]]
    _counts[_p[2]] += 1

PHASE_BUCKETS = {
    "n32": (-3, -2),
    "n1": (-1,),
    "z0": (0,),
    "p1": (1,),
    "p32": (2, 3),
}
PHASES = ("n32", "n1", "z0", "p1", "p32")


def _phase_of(m3):
    for name, ms in PHASE_BUCKETS.items():
        if m3 in ms:
            return name
    raise ValueError(m3)


def _cg_coeff(l1, m1, l2, m2, l3, m3):
    if m1 + m2 != m3:
        return 0.0
    pref = (
        (2 * l3 + 1)
        * fact(l3 + l1 - l2)
        * fact(l3 - l1 + l2)
        * fact(l1 + l2 - l3)
        / fact(l1 + l2 + l3 + 1)
    ) ** 0.5
    pref *= (
        fact(l3 + m3)
        * fact(l3 - m3)
        * fact(l1 - m1)
        * fact(l1 + m1)
        * fact(l2 - m2)
        * fact(l2 + m2)
    ) ** 0.5
    s = 0.0
    for k in range(
        max(0, l2 - l3 - m1, l1 - l3 + m2),
        min(l1 + l2 - l3, l1 - m1, l2 + m2) + 1,
    ):
        s += (-1) ** k / (
            fact(k)
            * fact(l1 + l2 - l3 - k)
            * fact(l1 - m1 - k)
            * fact(l2 + m2 - k)
            * fact(l3 - l2 + m1 + k)
            * fact(l3 - l1 - m2 + k)
        )
    return pref * s


def comp_idx(l, m):
    return l * l + (m + l)


def build_tables(coeff_tol=1e-9):
    band = {}
    for p in PATHS:
        l1, l2, l3 = p
        for m3 in range(-l3, l3 + 1):
            terms = []
            for m1 in range(-l1, l1 + 1):
                m2 = m3 - m1
                if -l2 <= m2 <= l2:
                    c = _cg_coeff(l1, m1, l2, m2, l3, m3)
                    if abs(c) > coeff_tol:
                        terms.append((m1, m2, c))
            if terms:
                band[(p, m3)] = terms

    variants = [("identity", 1.0)]
    variant_idx = {("identity", 1.0): 0}

    def get_variant(p, coeff):
        key = (p, round(coeff, 9))
        if key not in variant_idx:
            variant_idx[key] = len(variants)
            variants.append(key)
        return variant_idx[key]

    pairs = sorted({(p[0], p[1]) for p in PATHS})
    phases = {}
    for ph in PHASES:
        comps = []
        for l3 in range(L_MAX + 1):
            for m3 in range(-l3, l3 + 1):
                if _phase_of(m3) == ph and any(
                    (p, m3) in band for p in PATHS if p[2] == l3
                ):
                    comps.append((l3, m3))
        pair_entries = []
        comp_matmuls = {c: [] for c in comps}
        for pr in pairs:
            l1, l2 = pr
            pr_paths = [p for p in PATHS if (p[0], p[1]) == pr]
            used = set()
            for p in pr_paths:
                for m3 in range(-p[2], p[2] + 1):
                    if _phase_of(m3) != ph:
                        continue
                    for (m1, m2, c) in band.get((p, m3), []):
                        used.add((m1, m2))
            if not used:
                continue
            # zip products: for fixed m3, (m1, m3-m1) pairs align f1 comps
            # (ascending m1) with reversed-order f2 comps (ascending index).
            row_of = {}
            rows = []
            prod_ops = []  # (m3, m1_start, length, row_start)
            for m3 in sorted({m1 + m2 for (m1, m2) in used}):
                m1s = sorted(m1 for (m1, m2) in used if m1 + m2 == m3)
                m1_start, m1_end = m1s[0], m1s[-1]
                row_start = len(rows)
                for m1 in range(m1_start, m1_end + 1):
                    row_of[(m1, m3 - m1)] = len(rows)
                    rows.append((m1, m3 - m1))
                prod_ops.append((m3, m1_start, m1_end - m1_start + 1, row_start))
            # mirror folds (l1 == l2): within a zip, c(m3-m1) = (-1)^l3 c(m1),
            # so rows i and L-1-i combine as row_i +/- row_{L-1-i} for all
            # bands of one l3-parity -- one DVE op per (zip, parity).
            zip_info = {m3: (m1s, ln, r0) for (m3, m1s, ln, r0) in prod_ops}
            fold_ops = []  # (row0, L, eps, frow0, h)
            fold_index = {}
            n_fold = 0
            if l1 == l2:
                need = set()
                for p in pr_paths:
                    l3 = p[2]
                    eps = 1 if l3 % 2 == 0 else -1
                    for m3 in range(-l3, l3 + 1):
                        if _phase_of(m3) != ph or (p, m3) not in band:
                            continue
                        m1s, ln, r0 = zip_info[m3]
                        h = ln // 2
                        for (m1, m2, c) in band[(p, m3)]:
                            i = m1 - m1s
                            if i < h:
                                cm = _cg_coeff(l1, m2, l2, m1, l3, m3)
                                assert abs(cm - eps * c) < 1e-9
                                need.add((m3, eps))
                for (m3, eps) in sorted(need, key=lambda t: (t[0], -t[1])):
                    m1s, ln, r0 = zip_info[m3]
                    h = ln // 2
                    fold_ops.append((r0, ln, eps, n_fold, h))
                    for i in range(h):
                        fold_index[(m3, eps, i)] = n_fold + i
                    n_fold += h

            gsum_ops = []  # members: (kind, row, relsign)
            pair_mms = []
            for p in pr_paths:
                l3 = p[2]
                eps = 1 if l3 % 2 == 0 else -1
                for m3 in range(-l3, l3 + 1):
                    if _phase_of(m3) != ph or (p, m3) not in band:
                        continue
                    terms = band[(p, m3)]
                    # reduce via folds
                    red = []  # (kind, row, c)
                    if l1 == l2:
                        m1s, ln, r0 = zip_info[m3]
                        h = ln // 2
                        for (m1, m2, c) in terms:
                            i = m1 - m1s
                            if i < h:
                                red.append(("fold", fold_index[(m3, eps, i)], c))
                            elif i == ln - 1 - i:
                                red.append(("prod", row_of[(m1, m2)], c))
                            # i > mirror: covered by fold
                    else:
                        red = [("prod", row_of[(m1, m2)], c) for (m1, m2, c) in terms]
                    gs = {}
                    for (kind, row, c) in red:
                        gs.setdefault(round(abs(c), 9), []).append(
                            (kind, row, 1.0 if c > 0 else -1.0)
                        )
                    for gamma, members in sorted(gs.items()):
                        sigma1 = members[0][2]
                        vi = get_variant(p, sigma1 * gamma)
                        if len(members) == 1:
                            kind, row, _ = members[0]
                            pair_mms.append(((l3, m3), vi, kind, row))
                        else:
                            gid = len(gsum_ops)
                            gsum_ops.append(
                                [(k, r, sg * sigma1) for (k, r, sg) in members]
                            )
                            pair_mms.append(((l3, m3), vi, "gsum", gid))
            pair_entries.append(
                dict(pair=pr, rows=rows, prod_ops=prod_ops, fold_ops=fold_ops,
                     n_fold=n_fold, gsum_ops=gsum_ops, mms=pair_mms)
            )
            for mm in pair_mms:
                comp_matmuls[mm[0]].append((len(pair_entries) - 1,) + mm[1:])
        phases[ph] = dict(comps=comps, pairs=pair_entries, comp_matmuls=comp_matmuls)

    return dict(variants=variants, phases=phases)


def build_weight_variants(W_list, variants):
    out = np.zeros((len(variants), K, K), dtype=np.float32)
    for i, (p, coeff) in enumerate(variants):
        if p == "identity":
            out[i] = np.eye(K, dtype=np.float32)
        else:
            b = PATH_BLOCK[p]
            out[i] = coeff * W_list[p[2]][b * K : (b + 1) * K, :]
    return out



def phase_variant_layout(tables):
    """Per-phase variant lists (global ids, identity excluded) + dram offsets."""
    layout = {}
    off = 0
    for ph in PHASES:
        phase = tables["phases"][ph]
        vids = sorted(
            {mm[1] for pe in phase["pairs"] for mm in pe["mms"]}
        )
        local = {v: i for i, v in enumerate(vids)}
        layout[ph] = dict(vids=vids, local=local, offset=off)
        off += len(vids)
    return layout, off


def build_nc(tables, n_local=N_LOCAL, mm_free=512, gp_pairs=()):
    wlayout, wtotal = phase_variant_layout(tables)
    nc = bacc.Bacc("TRN2", target_bir_lowering=False, debug=False)

    f1_d = nc.dram_tensor("f1", [128, 16, n_local], BF16, kind="ExternalInput")
    f2_d = nc.dram_tensor("f2", [128, 16, n_local], BF16, kind="ExternalInput")
    wid_d = nc.dram_tensor("wid", [128, 128], BF16, kind="ExternalInput")
    wv_d = nc.dram_tensor("wv", [128, wtotal * 128], BF16, kind="ExternalInput")
    out_d = nc.dram_tensor("out", [128, 16, n_local], F32, kind="ExternalOutput")

    n_halves = n_local // mm_free

    with tile.TileContext(nc) as tc:
        with (
            tc.tile_pool(name="inp", bufs=1) as inp_pool,
            tc.tile_pool(name="wpool", bufs=2) as w_pool,
            tc.tile_pool(name="widp", bufs=1) as wid_pool,
            tc.tile_pool(name="prod", bufs=2) as prod_pool,
            tc.tile_pool(name="fold", bufs=2) as fold_pool,
            tc.tile_pool(name="gsum", bufs=4) as gsum_pool,
            tc.tile_pool(name="stage", bufs=3) as stage_pool,
            tc.tile_pool(name="psum", bufs=8, space="PSUM") as psum_pool,
        ):
            wid = wid_pool.tile([128, 128], BF16, tag="wid")
            nc.sync.dma_start(wid[:], wid_d[:])
            f1t = []
            f2t = []
            for l in range(4):
                nm = 2 * l + 1
                t1 = inp_pool.tile([128, nm, n_local], BF16, tag=f"f1_{l}",
                                   name=f"f1_{l}")
                t2 = inp_pool.tile([128, nm, n_local], BF16, tag=f"f2_{l}",
                                   name=f"f2_{l}")
                nc.sync.dma_start(t1[:], f1_d[:, l * l : l * l + nm, :])
                nc.sync.dma_start(t2[:], f2_d[:, l * l : l * l + nm, :])
                f1t.append(t1)
                f2t.append(t2)

            for ph in PHASES:
                phase = tables["phases"][ph]
                comps = phase["comps"]
                lay = wlayout[ph]
                nvp = len(lay["vids"])
                wvp = w_pool.tile([128, nvp, 128], BF16, tag="wvp",
                                  name=f"wvp_{ph}")
                nc.sync.dma_start(
                    wvp[:],
                    wv_d[:, lay["offset"] * 128 : (lay["offset"] + nvp) * 128]
                    .rearrange("p (v k) -> p v k", v=nvp),
                )
                mm_total = {c: len(phase["comp_matmuls"][c]) for c in comps}
                mm_done = {c: 0 for c in comps}

                psum = {}
                for comp in comps:
                    for h in range(n_halves):
                        psum[(comp, h)] = psum_pool.tile(
                            [128, mm_free],
                            F32,
                            tag="ps",
                            name=f"ps_{ph}_{comp_idx(*comp)}_{h}",
                        )
                for comp in comps:
                    l3, m3 = comp
                    for h in range(n_halves):
                        sl = slice(h * mm_free, (h + 1) * mm_free)
                        nc.tensor.matmul(
                            psum[(comp, h)][:],
                            wid[:],
                            f1t[l3][:, m3 + l3, sl],
                            start=True,
                            stop=False,
                            skip_group_check=True,
                        )

                for pe in phase["pairs"]:
                    l1, l2 = pe["pair"]
                    rows = pe["rows"]
                    P = prod_pool.tile([128, len(rows), n_local], BF16, tag="P")
                    eng = nc.gpsimd if (l1, l2) in gp_pairs else nc.vector
                    for (m3, m1s, length, row0) in pe["prod_ops"]:
                        a0 = m1s + l1
                        # reversed f2: comp j = l2 - m2 = l2 - m3 + m1
                        j0 = l2 - m3 + m1s
                        eng.tensor_tensor(
                            P[:, row0 : row0 + length, :],
                            f1t[l1][:, a0 : a0 + length, :],
                            f2t[l2][:, j0 : j0 + length, :],
                            mybir.AluOpType.mult,
                        )
                    F = None
                    if pe["n_fold"]:
                        F = fold_pool.tile(
                            [128, pe["n_fold"], n_local], BF16, tag="F"
                        )
                        for (r0, ln, eps, f0, fh) in pe["fold_ops"]:
                            op = (
                                mybir.AluOpType.add
                                if eps > 0
                                else mybir.AluOpType.subtract
                            )
                            nc.vector.tensor_tensor(
                                F[:, f0 : f0 + fh, :],
                                P[:, r0 : r0 + fh, :],
                                P[:, r0 + ln - 1 : r0 + ln - 1 - fh : -1, :],
                                op,
                            )

                    def src(kind, row, sl=slice(None)):
                        t = P if kind == "prod" else F
                        return t[:, row, sl]

                    G = []
                    for members in pe["gsum_ops"]:
                        g = gsum_pool.tile([128, n_local], BF16, tag="G")
                        (k0, r0, s0), (k1, r1, s1) = members[0], members[1]
                        op = mybir.AluOpType.add if s1 > 0 else mybir.AluOpType.subtract
                        nc.vector.tensor_tensor(g[:], src(k0, r0), src(k1, r1), op)
                        for (k, r, sg) in members[2:]:
                            op = (
                                mybir.AluOpType.add
                                if sg > 0
                                else mybir.AluOpType.subtract
                            )
                            nc.vector.tensor_tensor(g[:], g[:], src(k, r), op)
                        G.append(g)

                    for (comp, vi, kind, ref) in pe["mms"]:
                        mm_done[comp] += 1
                        is_last = mm_done[comp] == mm_total[comp]
                        for h in range(n_halves):
                            sl = slice(h * mm_free, (h + 1) * mm_free)
                            rhs = G[ref][:, sl] if kind == "gsum" else src(kind, ref, sl)
                            nc.tensor.matmul(
                                psum[(comp, h)][:],
                                wvp[:, lay["local"][vi], :],
                                rhs,
                                start=False,
                                stop=is_last,
                                skip_group_check=True,
                            )
                        if is_last:
                            ci = comp_idx(*comp)
                            for h in range(n_halves):
                                sl = slice(h * mm_free, (h + 1) * mm_free)
                                st = stage_pool.tile([128, mm_free], F32, tag="stage")
                                nc.scalar.copy(st[:], psum[(comp, h)][:])
                                nc.sync.dma_start(out_d[:, ci, sl], st[:])

    nc.compile()
    return nc


_CACHE = {}


def _get_compiled():
    if "nc" not in _CACHE:
        tables = build_tables()
        _CACHE["tables"] = tables
        _CACHE["nc"] = build_nc(tables)
    return _CACHE["nc"], _CACHE["tables"]


def kernel(
    f1_l0, f1_l1, f1_l2, f1_l3,
    f2_l0, f2_l1, f2_l2, f2_l3,
    W_l0, W_l1, W_l2, W_l3,
    _trace=False,
):
    nc, tables = _get_compiled()

    f1_list = [np.asarray(f1_l0), np.asarray(f1_l1), np.asarray(f1_l2), np.asarray(f1_l3)]
    f2_list = [np.asarray(f2_l0), np.asarray(f2_l1), np.asarray(f2_l2), np.asarray(f2_l3)]
    W_list = [np.asarray(W_l0), np.asarray(W_l1), np.asarray(W_l2), np.asarray(W_l3)]

    def pack(fl, reverse_m=False):
        if reverse_m:
            fl = [f[:, ::-1, :] for f in fl]
        comps = np.concatenate(fl, axis=1)  # [N, 16, K] f32
        # -> [K, 16, N] bf16
        return np.ascontiguousarray(comps.transpose(2, 1, 0)).astype(ml_dtypes.bfloat16)

    F1 = pack(f1_list)
    F2 = pack(f2_list, reverse_m=True)
    WVfull = build_weight_variants(W_list, tables["variants"])  # [NV,128,128]
    wlayout, wtotal = phase_variant_layout(tables)
    WVp = np.zeros((wtotal, K, K), dtype=np.float32)
    for ph in PHASES:
        lay = wlayout[ph]
        for i, v in enumerate(lay["vids"]):
            WVp[lay["offset"] + i] = WVfull[v]
    WV = np.ascontiguousarray(
        WVp.astype(ml_dtypes.bfloat16).transpose(1, 0, 2).reshape(K, -1)
    )
    WID = np.ascontiguousarray(np.eye(K, dtype=np.float32).astype(ml_dtypes.bfloat16))

    in_maps = []
    for c in range(N_CORES):
        sl = slice(c * N_LOCAL, (c + 1) * N_LOCAL)
        in_maps.append(
            {
                "f1": np.ascontiguousarray(F1[:, :, sl]),
                "f2": np.ascontiguousarray(F2[:, :, sl]),
                "wv": WV,
                "wid": WID,
            }
        )

    res = run_bass_kernel_spmd(
        nc, in_maps, core_ids=list(range(N_CORES)), trace=_trace
    )
    _CACHE["last_result"] = res

    out_full = np.concatenate(
        [res.results[c]["out"] for c in range(N_CORES)], axis=2
    )  # [K, 16, N]
    outs = []
    offs = [0, 1, 4, 9, 16]
    for l in range(4):
        outs.append(
            np.ascontiguousarray(
                out_full[:, offs[l] : offs[l + 1], :].transpose(2, 1, 0)
            ).astype(np.float32)
        )
    return tuple(outs)


# revision 11
# speedup vs baseline: 1.1449x; 1.1449x over previous
"""Trainium2 Bass kernel for the CG tensor-product iteration (nn_CGIteration).

out[l3] = f1[l3] + concat_paths(einsum('abm,nak,nbk->nmk', C_p, f1[l1], f2[l2])) @ W[l3]

Self-contained: CG coefficients and the device schedule are computed here.
Data-parallel over nodes N: each of the 8 NeuronCores processes N/8 nodes.

Device algorithm (channel-major layout [k'=128 partitions, comp, n]):
  - products  P[m1,m2] = f1[l1,m1] * f2[l2,m2]   (DVE bf16 tensor_tensor,
    several (m1, m2-run) components per instruction via broadcast APs)
  - equal-|c| group sums S_g = sum_i sign_i P_i   (DVE add/sub)
  - PE matmuls psum[l3,m3] += (sign*|c| * W_path).T @ S_g with PSUM
    accumulation; the residual f1[l3] enters as an identity matmul
  - ACT evicts psum -> SBUF f32, DMA to DRAM
Weight variants (coeff * W_path, bf16) are prescaled on the host and DMAed.
"""

import sys
from math import factorial as fact

import numpy as np

if "/opt/trn_rl_repo" not in sys.path:  # harness safety; usually importable
    sys.path.append("/opt/trn_rl_repo")

import ml_dtypes

import concourse.mybir as mybir
import concourse.tile as tile
from concourse import bacc
from concourse.bass_utils import run_bass_kernel_spmd

BF16 = mybir.dt.bfloat16
F32 = mybir.dt.float32

L_MAX = 3
K = 128
N_TOTAL = 8192
N_CORES = 8
N_LOCAL = N_TOTAL // N_CORES

PATHS = [
    (l1, l2, l3)
    for l1 in range(L_MAX + 1)
    for l2 in range(L_MAX + 1)
    for l3 in range(abs(l1 - l2), min(l1 + l2, L_MAX) + 1)
]

PATH_BLOCK = {}
_counts = [0, 0, 0, 0]
for _p in PATHS:
    PATH_BLOCK[_p] = _counts[_p[2]]
    _counts[_p[2]] += 1

PHASE_BUCKETS = {
    "n32": (-3, -2),
    "n1": (-1,),
    "z0": (0,),
    "p1": (1,),
    "p32": (2, 3),
}
PHASES = ("n32", "n1", "z0", "p1", "p32")


def _phase_of(m3):
    for name, ms in PHASE_BUCKETS.items():
        if m3 in ms:
            return name
    raise ValueError(m3)


def _cg_coeff(l1, m1, l2, m2, l3, m3):
    if m1 + m2 != m3:
        return 0.0
    pref = (
        (2 * l3 + 1)
        * fact(l3 + l1 - l2)
        * fact(l3 - l1 + l2)
        * fact(l1 + l2 - l3)
        / fact(l1 + l2 + l3 + 1)
    ) ** 0.5
    pref *= (
        fact(l3 + m3)
        * fact(l3 - m3)
        * fact(l1 - m1)
        * fact(l1 + m1)
        * fact(l2 - m2)
        * fact(l2 + m2)
    ) ** 0.5
    s = 0.0
    for k in range(
        max(0, l2 - l3 - m1, l1 - l3 + m2),
        min(l1 + l2 - l3, l1 - m1, l2 + m2) + 1,
    ):
        s += (-1) ** k / (
            fact(k)
            * fact(l1 + l2 - l3 - k)
            * fact(l1 - m1 - k)
            * fact(l2 + m2 - k)
            * fact(l3 - l2 + m1 + k)
            * fact(l3 - l1 - m2 + k)
        )
    return pref * s


def comp_idx(l, m):
    return l * l + (m + l)


def build_tables(coeff_tol=1e-9):
    band = {}
    for p in PATHS:
        l1, l2, l3 = p
        for m3 in range(-l3, l3 + 1):
            terms = []
            for m1 in range(-l1, l1 + 1):
                m2 = m3 - m1
                if -l2 <= m2 <= l2:
                    c = _cg_coeff(l1, m1, l2, m2, l3, m3)
                    if abs(c) > coeff_tol:
                        terms.append((m1, m2, c))
            if terms:
                band[(p, m3)] = terms

    variants = [("identity", 1.0)]
    variant_idx = {("identity", 1.0): 0}

    def get_variant(p, coeff):
        key = (p, round(coeff, 9))
        if key not in variant_idx:
            variant_idx[key] = len(variants)
            variants.append(key)
        return variant_idx[key]

    pairs = sorted({(p[0], p[1]) for p in PATHS})
    phases = {}
    for ph in PHASES:
        comps = []
        for l3 in range(L_MAX + 1):
            for m3 in range(-l3, l3 + 1):
                if _phase_of(m3) == ph and any(
                    (p, m3) in band for p in PATHS if p[2] == l3
                ):
                    comps.append((l3, m3))
        pair_entries = []
        comp_matmuls = {c: [] for c in comps}
        for pr in pairs:
            l1, l2 = pr
            pr_paths = [p for p in PATHS if (p[0], p[1]) == pr]
            used = set()
            for p in pr_paths:
                for m3 in range(-p[2], p[2] + 1):
                    if _phase_of(m3) != ph:
                        continue
                    for (m1, m2, c) in band.get((p, m3), []):
                        used.add((m1, m2))
            if not used:
                continue
            # zip products: for fixed m3, (m1, m3-m1) pairs align f1 comps
            # (ascending m1) with reversed-order f2 comps (ascending index).
            row_of = {}
            rows = []
            prod_ops = []  # (m3, m1_start, length, row_start)
            for m3 in sorted({m1 + m2 for (m1, m2) in used}):
                m1s = sorted(m1 for (m1, m2) in used if m1 + m2 == m3)
                m1_start, m1_end = m1s[0], m1s[-1]
                row_start = len(rows)
                for m1 in range(m1_start, m1_end + 1):
                    row_of[(m1, m3 - m1)] = len(rows)
                    rows.append((m1, m3 - m1))
                prod_ops.append((m3, m1_start, m1_end - m1_start + 1, row_start))
            # mirror folds (l1 == l2): within a zip, c(m3-m1) = (-1)^l3 c(m1),
            # so rows i and L-1-i combine as row_i +/- row_{L-1-i} for all
            # bands of one l3-parity -- one DVE op per (zip, parity).
            zip_info = {m3: (m1s, ln, r0) for (m3, m1s, ln, r0) in prod_ops}
            fold_ops = []  # (row0, L, eps, frow0, h)
            fold_index = {}
            n_fold = 0
            if l1 == l2:
                need = set()
                for p in pr_paths:
                    l3 = p[2]
                    eps = 1 if l3 % 2 == 0 else -1
                    for m3 in range(-l3, l3 + 1):
                        if _phase_of(m3) != ph or (p, m3) not in band:
                            continue
                        m1s, ln, r0 = zip_info[m3]
                        h = ln // 2
                        for (m1, m2, c) in band[(p, m3)]:
                            i = m1 - m1s
                            if i < h:
                                cm = _cg_coeff(l1, m2, l2, m1, l3, m3)
                                assert abs(cm - eps * c) < 1e-9
                                need.add((m3, eps))
                for (m3, eps) in sorted(need, key=lambda t: (t[0], -t[1])):
                    m1s, ln, r0 = zip_info[m3]
                    h = ln // 2
                    fold_ops.append((r0, ln, eps, n_fold, h))
                    for i in range(h):
                        fold_index[(m3, eps, i)] = n_fold + i
                    n_fold += h

            gsum_ops = []  # members: (kind, row, relsign)
            pair_mms = []
            for p in pr_paths:
                l3 = p[2]
                eps = 1 if l3 % 2 == 0 else -1
                for m3 in range(-l3, l3 + 1):
                    if _phase_of(m3) != ph or (p, m3) not in band:
                        continue
                    terms = band[(p, m3)]
                    # reduce via folds
                    red = []  # (kind, row, c)
                    if l1 == l2:
                        m1s, ln, r0 = zip_info[m3]
                        h = ln // 2
                        for (m1, m2, c) in terms:
                            i = m1 - m1s
                            if i < h:
                                red.append(("fold", fold_index[(m3, eps, i)], c))
                            elif i == ln - 1 - i:
                                red.append(("prod", row_of[(m1, m2)], c))
                            # i > mirror: covered by fold
                    else:
                        red = [("prod", row_of[(m1, m2)], c) for (m1, m2, c) in terms]
                    gs = {}
                    for (kind, row, c) in red:
                        gs.setdefault(round(abs(c), 9), []).append(
                            (kind, row, 1.0 if c > 0 else -1.0)
                        )
                    for gamma, members in sorted(gs.items()):
                        sigma1 = members[0][2]
                        vi = get_variant(p, sigma1 * gamma)
                        if len(members) == 1:
                            kind, row, _ = members[0]
                            pair_mms.append(((l3, m3), vi, kind, row))
                        else:
                            gid = len(gsum_ops)
                            gsum_ops.append(
                                [(k, r, sg * sigma1) for (k, r, sg) in members]
                            )
                            pair_mms.append(((l3, m3), vi, "gsum", gid))
            pair_entries.append(
                dict(pair=pr, rows=rows, prod_ops=prod_ops, fold_ops=fold_ops,
                     n_fold=n_fold, gsum_ops=gsum_ops, mms=pair_mms)
            )
            for mm in pair_mms:
                comp_matmuls[mm[0]].append((len(pair_entries) - 1,) + mm[1:])
        phases[ph] = dict(comps=comps, pairs=pair_entries, comp_matmuls=comp_matmuls)

    return dict(variants=variants, phases=phases)


def build_weight_variants(W_list, variants):
    out = np.zeros((len(variants), K, K), dtype=np.float32)
    for i, (p, coeff) in enumerate(variants):
        if p == "identity":
            out[i] = np.eye(K, dtype=np.float32)
        else:
            b = PATH_BLOCK[p]
            out[i] = coeff * W_list[p[2]][b * K : (b + 1) * K, :]
    return out



def phase_variant_layout(tables):
    """Per-phase variant lists (global ids, identity excluded) + dram offsets."""
    layout = {}
    off = 0
    for ph in PHASES:
        phase = tables["phases"][ph]
        vids = sorted(
            {mm[1] for pe in phase["pairs"] for mm in pe["mms"]}
        )
        local = {v: i for i, v in enumerate(vids)}
        layout[ph] = dict(vids=vids, local=local, offset=off)
        off += len(vids)
    return layout, off


def build_nc(tables, n_local=N_LOCAL, mm_free=512, gp_pairs=()):
    wlayout, wtotal = phase_variant_layout(tables)
    nc = bacc.Bacc("TRN2", target_bir_lowering=False, debug=False)

    f1_d = nc.dram_tensor("f1", [128, 16, n_local], BF16, kind="ExternalInput")
    f2_d = nc.dram_tensor("f2", [128, 16, n_local], BF16, kind="ExternalInput")
    wid_d = nc.dram_tensor("wid", [128, 128], BF16, kind="ExternalInput")
    wv_d = nc.dram_tensor("wv", [128, wtotal * 128], BF16, kind="ExternalInput")
    out_d = nc.dram_tensor("out", [128, 16, n_local], F32, kind="ExternalOutput")

    n_halves = n_local // mm_free

    with tile.TileContext(nc) as tc:
        with (
            tc.tile_pool(name="inp", bufs=1) as inp_pool,
            tc.tile_pool(name="wpool", bufs=2) as w_pool,
            tc.tile_pool(name="widp", bufs=1) as wid_pool,
            tc.tile_pool(name="prod", bufs=3) as prod_pool,
            tc.tile_pool(name="fold", bufs=1) as fold_pool,
            tc.tile_pool(name="gsum", bufs=4) as gsum_pool,
            tc.tile_pool(name="stage", bufs=3) as stage_pool,
            tc.tile_pool(name="psum", bufs=8, space="PSUM") as psum_pool,
        ):
            wid = wid_pool.tile([128, 128], BF16, tag="wid")
            nc.sync.dma_start(wid[:], wid_d[:])
            f1t = []
            f2t = []
            for l in range(4):
                nm = 2 * l + 1
                t1 = inp_pool.tile([128, nm, n_local], BF16, tag=f"f1_{l}",
                                   name=f"f1_{l}")
                t2 = inp_pool.tile([128, nm, n_local], BF16, tag=f"f2_{l}",
                                   name=f"f2_{l}")
                nc.sync.dma_start(t1[:], f1_d[:, l * l : l * l + nm, :])
                nc.sync.dma_start(t2[:], f2_d[:, l * l : l * l + nm, :])
                f1t.append(t1)
                f2t.append(t2)

            for ph in PHASES:
                phase = tables["phases"][ph]
                comps = phase["comps"]
                lay = wlayout[ph]
                nvp = len(lay["vids"])
                wvp = w_pool.tile([128, nvp, 128], BF16, tag="wvp",
                                  name=f"wvp_{ph}")
                nc.sync.dma_start(
                    wvp[:],
                    wv_d[:, lay["offset"] * 128 : (lay["offset"] + nvp) * 128]
                    .rearrange("p (v k) -> p v k", v=nvp),
                )
                mm_total = {c: len(phase["comp_matmuls"][c]) for c in comps}
                mm_done = {c: 0 for c in comps}

                psum = {}
                for comp in comps:
                    for h in range(n_halves):
                        psum[(comp, h)] = psum_pool.tile(
                            [128, mm_free],
                            F32,
                            tag="ps",
                            name=f"ps_{ph}_{comp_idx(*comp)}_{h}",
                        )
                for comp in comps:
                    l3, m3 = comp
                    for h in range(n_halves):
                        sl = slice(h * mm_free, (h + 1) * mm_free)
                        nc.tensor.matmul(
                            psum[(comp, h)][:],
                            wid[:],
                            f1t[l3][:, m3 + l3, sl],
                            start=True,
                            stop=False,
                            skip_group_check=True,
                        )

                for pe in phase["pairs"]:
                    l1, l2 = pe["pair"]
                    rows = pe["rows"]
                    P = prod_pool.tile([128, len(rows), n_local], BF16, tag="P")
                    eng = nc.gpsimd if (l1, l2) in gp_pairs else nc.vector
                    for (m3, m1s, length, row0) in pe["prod_ops"]:
                        a0 = m1s + l1
                        # reversed f2: comp j = l2 - m2 = l2 - m3 + m1
                        j0 = l2 - m3 + m1s
                        eng.tensor_tensor(
                            P[:, row0 : row0 + length, :],
                            f1t[l1][:, a0 : a0 + length, :],
                            f2t[l2][:, j0 : j0 + length, :],
                            mybir.AluOpType.mult,
                        )
                    F = None
                    if pe["n_fold"]:
                        F = fold_pool.tile(
                            [128, pe["n_fold"], n_local], BF16, tag="F"
                        )
                        for (r0, ln, eps, f0, fh) in pe["fold_ops"]:
                            op = (
                                mybir.AluOpType.add
                                if eps > 0
                                else mybir.AluOpType.subtract
                            )
                            nc.vector.tensor_tensor(
                                F[:, f0 : f0 + fh, :],
                                P[:, r0 : r0 + fh, :],
                                P[:, r0 + ln - 1 : r0 + ln - 1 - fh : -1, :],
                                op,
                            )

                    def src(kind, row, sl=slice(None)):
                        t = P if kind == "prod" else F
                        return t[:, row, sl]

                    G = []
                    for members in pe["gsum_ops"]:
                        g = gsum_pool.tile([128, n_local], BF16, tag="G")
                        (k0, r0, s0), (k1, r1, s1) = members[0], members[1]
                        op = mybir.AluOpType.add if s1 > 0 else mybir.AluOpType.subtract
                        nc.vector.tensor_tensor(g[:], src(k0, r0), src(k1, r1), op)
                        for (k, r, sg) in members[2:]:
                            op = (
                                mybir.AluOpType.add
                                if sg > 0
                                else mybir.AluOpType.subtract
                            )
                            nc.vector.tensor_tensor(g[:], g[:], src(k, r), op)
                        G.append(g)

                    for (comp, vi, kind, ref) in pe["mms"]:
                        mm_done[comp] += 1
                        is_last = mm_done[comp] == mm_total[comp]
                        for h in range(n_halves):
                            sl = slice(h * mm_free, (h + 1) * mm_free)
                            rhs = G[ref][:, sl] if kind == "gsum" else src(kind, ref, sl)
                            nc.tensor.matmul(
                                psum[(comp, h)][:],
                                wvp[:, lay["local"][vi], :],
                                rhs,
                                start=False,
                                stop=is_last,
                                skip_group_check=True,
                            )
                        if is_last:
                            ci = comp_idx(*comp)
                            for h in range(n_halves):
                                sl = slice(h * mm_free, (h + 1) * mm_free)
                                st = stage_pool.tile([128, mm_free], F32, tag="stage")
                                nc.scalar.copy(st[:], psum[(comp, h)][:])
                                nc.sync.dma_start(out_d[:, ci, sl], st[:])

    nc.compile()
    return nc


_CACHE = {}


def _get_compiled():
    if "nc" not in _CACHE:
        tables = build_tables()
        _CACHE["tables"] = tables
        _CACHE["nc"] = build_nc(tables)
    return _CACHE["nc"], _CACHE["tables"]


def kernel(
    f1_l0, f1_l1, f1_l2, f1_l3,
    f2_l0, f2_l1, f2_l2, f2_l3,
    W_l0, W_l1, W_l2, W_l3,
    _trace=False,
):
    nc, tables = _get_compiled()

    f1_list = [np.asarray(f1_l0), np.asarray(f1_l1), np.asarray(f1_l2), np.asarray(f1_l3)]
    f2_list = [np.asarray(f2_l0), np.asarray(f2_l1), np.asarray(f2_l2), np.asarray(f2_l3)]
    W_list = [np.asarray(W_l0), np.asarray(W_l1), np.asarray(W_l2), np.asarray(W_l3)]

    def pack(fl, reverse_m=False):
        if reverse_m:
            fl = [f[:, ::-1, :] for f in fl]
        comps = np.concatenate(fl, axis=1)  # [N, 16, K] f32
        # -> [K, 16, N] bf16
        return np.ascontiguousarray(comps.transpose(2, 1, 0)).astype(ml_dtypes.bfloat16)

    F1 = pack(f1_list)
    F2 = pack(f2_list, reverse_m=True)
    WVfull = build_weight_variants(W_list, tables["variants"])  # [NV,128,128]
    wlayout, wtotal = phase_variant_layout(tables)
    WVp = np.zeros((wtotal, K, K), dtype=np.float32)
    for ph in PHASES:
        lay = wlayout[ph]
        for i, v in enumerate(lay["vids"]):
            WVp[lay["offset"] + i] = WVfull[v]
    WV = np.ascontiguousarray(
        WVp.astype(ml_dtypes.bfloat16).transpose(1, 0, 2).reshape(K, -1)
    )
    WID = np.ascontiguousarray(np.eye(K, dtype=np.float32).astype(ml_dtypes.bfloat16))

    in_maps = []
    for c in range(N_CORES):
        sl = slice(c * N_LOCAL, (c + 1) * N_LOCAL)
        in_maps.append(
            {
                "f1": np.ascontiguousarray(F1[:, :, sl]),
                "f2": np.ascontiguousarray(F2[:, :, sl]),
                "wv": WV,
                "wid": WID,
            }
        )

    res = run_bass_kernel_spmd(
        nc, in_maps, core_ids=list(range(N_CORES)), trace=_trace
    )
    _CACHE["last_result"] = res

    out_full = np.concatenate(
        [res.results[c]["out"] for c in range(N_CORES)], axis=2
    )  # [K, 16, N]
    outs = []
    offs = [0, 1, 4, 9, 16]
    for l in range(4):
        outs.append(
            np.ascontiguousarray(
                out_full[:, offs[l] : offs[l + 1], :].transpose(2, 1, 0)
            ).astype(np.float32)
        )
    return tuple(outs)


# revision 12
# speedup vs baseline: 1.1580x; 1.0114x over previous
"""Trainium2 Bass kernel for the CG tensor-product iteration (nn_CGIteration).

out[l3] = f1[l3] + concat_paths(einsum('abm,nak,nbk->nmk', C_p, f1[l1], f2[l2])) @ W[l3]

Self-contained: CG coefficients and the device schedule are computed here.
Data-parallel over nodes N: each of the 8 NeuronCores processes N/8 nodes.

Device algorithm (channel-major layout [k'=128 partitions, comp, n]):
  - products  P[m1,m2] = f1[l1,m1] * f2[l2,m2]   (DVE bf16 tensor_tensor,
    several (m1, m2-run) components per instruction via broadcast APs)
  - equal-|c| group sums S_g = sum_i sign_i P_i   (DVE add/sub)
  - PE matmuls psum[l3,m3] += (sign*|c| * W_path).T @ S_g with PSUM
    accumulation; the residual f1[l3] enters as an identity matmul
  - ACT evicts psum -> SBUF f32, DMA to DRAM
Weight variants (coeff * W_path, bf16) are prescaled on the host and DMAed.
"""

import sys
from math import factorial as fact

import numpy as np

if "/opt/trn_rl_repo" not in sys.path:  # harness safety; usually importable
    sys.path.append("/opt/trn_rl_repo")

import ml_dtypes

import concourse.mybir as mybir
import concourse.tile as tile
from concourse import bacc
from concourse.bass_utils import run_bass_kernel_spmd

BF16 = mybir.dt.bfloat16
F32 = mybir.dt.float32

L_MAX = 3
K = 128
N_TOTAL = 8192
N_CORES = 8
N_LOCAL = N_TOTAL // N_CORES

PATHS = [
    (l1, l2, l3)
    for l1 in range(L_MAX + 1)
    for l2 in range(L_MAX + 1)
    for l3 in range(abs(l1 - l2), min(l1 + l2, L_MAX) + 1)
]

PATH_BLOCK = {}
_counts = [0, 0, 0, 0]
for _p in PATHS:
    PATH_BLOCK[_p] = _counts[_p[2]]
    _counts[_p[2]] += 1

PHASE_BUCKETS = {
    "n32": (-3, -2),
    "n1": (-1,),
    "z0": (0,),
    "p1": (1,),
    "p32": (2, 3),
}
PHASES = ("n32", "n1", "z0", "p1", "p32")


def _phase_of(m3):
    for name, ms in PHASE_BUCKETS.items():
        if m3 in ms:
            return name
    raise ValueError(m3)


def _cg_coeff(l1, m1, l2, m2, l3, m3):
    if m1 + m2 != m3:
        return 0.0
    pref = (
        (2 * l3 + 1)
        * fact(l3 + l1 - l2)
        * fact(l3 - l1 + l2)
        * fact(l1 + l2 - l3)
        / fact(l1 + l2 + l3 + 1)
    ) ** 0.5
    pref *= (
        fact(l3 + m3)
        * fact(l3 - m3)
        * fact(l1 - m1)
        * fact(l1 + m1)
        * fact(l2 - m2)
        * fact(l2 + m2)
    ) ** 0.5
    s = 0.0
    for k in range(
        max(0, l2 - l3 - m1, l1 - l3 + m2),
        min(l1 + l2 - l3, l1 - m1, l2 + m2) + 1,
    ):
        s += (-1) ** k / (
            fact(k)
            * fact(l1 + l2 - l3 - k)
            * fact(l1 - m1 - k)
            * fact(l2 + m2 - k)
            * fact(l3 - l2 + m1 + k)
            * fact(l3 - l1 - m2 + k)
        )
    return pref * s


def comp_idx(l, m):
    return l * l + (m + l)


def build_tables(coeff_tol=1e-9):
    band = {}
    for p in PATHS:
        l1, l2, l3 = p
        for m3 in range(-l3, l3 + 1):
            terms = []
            for m1 in range(-l1, l1 + 1):
                m2 = m3 - m1
                if -l2 <= m2 <= l2:
                    c = _cg_coeff(l1, m1, l2, m2, l3, m3)
                    if abs(c) > coeff_tol:
                        terms.append((m1, m2, c))
            if terms:
                band[(p, m3)] = terms

    variants = [("identity", 1.0)]
    variant_idx = {("identity", 1.0): 0}

    def get_variant(p, coeff):
        key = (p, round(coeff, 9))
        if key not in variant_idx:
            variant_idx[key] = len(variants)
            variants.append(key)
        return variant_idx[key]

    pairs = sorted({(p[0], p[1]) for p in PATHS})
    phases = {}
    for ph in PHASES:
        comps = []
        for l3 in range(L_MAX + 1):
            for m3 in range(-l3, l3 + 1):
                if _phase_of(m3) == ph and any(
                    (p, m3) in band for p in PATHS if p[2] == l3
                ):
                    comps.append((l3, m3))
        pair_entries = []
        comp_matmuls = {c: [] for c in comps}
        for pr in pairs:
            l1, l2 = pr
            pr_paths = [p for p in PATHS if (p[0], p[1]) == pr]
            used = set()
            for p in pr_paths:
                for m3 in range(-p[2], p[2] + 1):
                    if _phase_of(m3) != ph:
                        continue
                    for (m1, m2, c) in band.get((p, m3), []):
                        used.add((m1, m2))
            if not used:
                continue
            # zip products: for fixed m3, (m1, m3-m1) pairs align f1 comps
            # (ascending m1) with reversed-order f2 comps (ascending index).
            row_of = {}
            rows = []
            prod_ops = []  # (m3, m1_start, length, row_start)
            for m3 in sorted({m1 + m2 for (m1, m2) in used}):
                m1s = sorted(m1 for (m1, m2) in used if m1 + m2 == m3)
                m1_start, m1_end = m1s[0], m1s[-1]
                row_start = len(rows)
                for m1 in range(m1_start, m1_end + 1):
                    row_of[(m1, m3 - m1)] = len(rows)
                    rows.append((m1, m3 - m1))
                prod_ops.append((m3, m1_start, m1_end - m1_start + 1, row_start))
            # mirror folds (l1 == l2): within a zip, c(m3-m1) = (-1)^l3 c(m1),
            # so rows i and L-1-i combine as row_i +/- row_{L-1-i} for all
            # bands of one l3-parity -- one DVE op per (zip, parity).
            zip_info = {m3: (m1s, ln, r0) for (m3, m1s, ln, r0) in prod_ops}
            fold_ops = []  # (row0, L, eps, frow0, h)
            fold_index = {}
            n_fold = 0
            if l1 == l2:
                need = set()
                for p in pr_paths:
                    l3 = p[2]
                    eps = 1 if l3 % 2 == 0 else -1
                    for m3 in range(-l3, l3 + 1):
                        if _phase_of(m3) != ph or (p, m3) not in band:
                            continue
                        m1s, ln, r0 = zip_info[m3]
                        h = ln // 2
                        for (m1, m2, c) in band[(p, m3)]:
                            i = m1 - m1s
                            if i < h:
                                cm = _cg_coeff(l1, m2, l2, m1, l3, m3)
                                assert abs(cm - eps * c) < 1e-9
                                need.add((m3, eps))
                for (m3, eps) in sorted(need, key=lambda t: (t[0], -t[1])):
                    m1s, ln, r0 = zip_info[m3]
                    h = ln // 2
                    fold_ops.append((r0, ln, eps, n_fold, h))
                    for i in range(h):
                        fold_index[(m3, eps, i)] = n_fold + i
                    n_fold += h

            gsum_ops = []  # members: (kind, row, relsign)
            pair_mms = []
            for p in pr_paths:
                l3 = p[2]
                eps = 1 if l3 % 2 == 0 else -1
                for m3 in range(-l3, l3 + 1):
                    if _phase_of(m3) != ph or (p, m3) not in band:
                        continue
                    terms = band[(p, m3)]
                    # reduce via folds
                    red = []  # (kind, row, c)
                    if l1 == l2:
                        m1s, ln, r0 = zip_info[m3]
                        h = ln // 2
                        for (m1, m2, c) in terms:
                            i = m1 - m1s
                            if i < h:
                                red.append(("fold", fold_index[(m3, eps, i)], c))
                            elif i == ln - 1 - i:
                                red.append(("prod", row_of[(m1, m2)], c))
                            # i > mirror: covered by fold
                    else:
                        red = [("prod", row_of[(m1, m2)], c) for (m1, m2, c) in terms]
                    gs = {}
                    for (kind, row, c) in red:
                        gs.setdefault(round(abs(c), 9), []).append(
                            (kind, row, 1.0 if c > 0 else -1.0)
                        )
                    for gamma, members in sorted(gs.items()):
                        sigma1 = members[0][2]
                        vi = get_variant(p, sigma1 * gamma)
                        if len(members) == 1:
                            kind, row, _ = members[0]
                            pair_mms.append(((l3, m3), vi, kind, row))
                        else:
                            gid = len(gsum_ops)
                            gsum_ops.append(
                                [(k, r, sg * sigma1) for (k, r, sg) in members]
                            )
                            pair_mms.append(((l3, m3), vi, "gsum", gid))
            pair_entries.append(
                dict(pair=pr, rows=rows, prod_ops=prod_ops, fold_ops=fold_ops,
                     n_fold=n_fold, gsum_ops=gsum_ops, mms=pair_mms)
            )
            for mm in pair_mms:
                comp_matmuls[mm[0]].append((len(pair_entries) - 1,) + mm[1:])
        phases[ph] = dict(comps=comps, pairs=pair_entries, comp_matmuls=comp_matmuls)

    return dict(variants=variants, phases=phases)


def build_weight_variants(W_list, variants):
    out = np.zeros((len(variants), K, K), dtype=np.float32)
    for i, (p, coeff) in enumerate(variants):
        if p == "identity":
            out[i] = np.eye(K, dtype=np.float32)
        else:
            b = PATH_BLOCK[p]
            out[i] = coeff * W_list[p[2]][b * K : (b + 1) * K, :]
    return out



def phase_variant_layout(tables):
    """Per-phase variant lists (global ids, identity excluded) + dram offsets."""
    layout = {}
    off = 0
    for ph in PHASES:
        phase = tables["phases"][ph]
        vids = sorted(
            {mm[1] for pe in phase["pairs"] for mm in pe["mms"]}
        )
        local = {v: i for i, v in enumerate(vids)}
        layout[ph] = dict(vids=vids, local=local, offset=off)
        off += len(vids)
    return layout, off


def build_nc(tables, n_local=N_LOCAL, mm_free=512, gp_pairs=()):
    wlayout, wtotal = phase_variant_layout(tables)
    nc = bacc.Bacc("TRN2", target_bir_lowering=False, debug=False)

    f1_d = nc.dram_tensor("f1", [128, 16, n_local], BF16, kind="ExternalInput")
    f2_d = nc.dram_tensor("f2", [128, 16, n_local], BF16, kind="ExternalInput")
    wid_d = nc.dram_tensor("wid", [128, 128], BF16, kind="ExternalInput")
    wv_d = nc.dram_tensor("wv", [128, wtotal * 128], BF16, kind="ExternalInput")
    out_d = nc.dram_tensor("out", [128, 16, n_local], F32, kind="ExternalOutput")

    n_halves = n_local // mm_free

    with tile.TileContext(nc) as tc:
        with (
            tc.tile_pool(name="inp", bufs=1) as inp_pool,
            tc.tile_pool(name="wpool", bufs=2) as w_pool,
            tc.tile_pool(name="widp", bufs=1) as wid_pool,
            tc.tile_pool(name="prod", bufs=3) as prod_pool,
            tc.tile_pool(name="fold", bufs=2) as fold_pool,
            tc.tile_pool(name="gsum", bufs=6) as gsum_pool,
            tc.tile_pool(name="stage", bufs=4) as stage_pool,
            tc.tile_pool(name="psum", bufs=8, space="PSUM") as psum_pool,
        ):
            wid = wid_pool.tile([128, 128], BF16, tag="wid")
            nc.sync.dma_start(wid[:], wid_d[:])
            f1t = []
            f2t = []
            for l in range(4):
                nm = 2 * l + 1
                t1 = inp_pool.tile([128, nm, n_local], BF16, tag=f"f1_{l}",
                                   name=f"f1_{l}")
                t2 = inp_pool.tile([128, nm, n_local], BF16, tag=f"f2_{l}",
                                   name=f"f2_{l}")
                nc.sync.dma_start(t1[:], f1_d[:, l * l : l * l + nm, :])
                nc.sync.dma_start(t2[:], f2_d[:, l * l : l * l + nm, :])
                f1t.append(t1)
                f2t.append(t2)

            for ph in PHASES:
                phase = tables["phases"][ph]
                comps = phase["comps"]
                lay = wlayout[ph]
                nvp = len(lay["vids"])
                wvp = w_pool.tile([128, nvp, 128], BF16, tag="wvp",
                                  name=f"wvp_{ph}")
                nc.sync.dma_start(
                    wvp[:],
                    wv_d[:, lay["offset"] * 128 : (lay["offset"] + nvp) * 128]
                    .rearrange("p (v k) -> p v k", v=nvp),
                )
                mm_total = {c: len(phase["comp_matmuls"][c]) for c in comps}
                mm_done = {c: 0 for c in comps}

                psum = {}
                for comp in comps:
                    for h in range(n_halves):
                        psum[(comp, h)] = psum_pool.tile(
                            [128, mm_free],
                            F32,
                            tag="ps",
                            name=f"ps_{ph}_{comp_idx(*comp)}_{h}",
                        )
                for comp in comps:
                    l3, m3 = comp
                    for h in range(n_halves):
                        sl = slice(h * mm_free, (h + 1) * mm_free)
                        nc.tensor.matmul(
                            psum[(comp, h)][:],
                            wid[:],
                            f1t[l3][:, m3 + l3, sl],
                            start=True,
                            stop=False,
                            skip_group_check=True,
                        )

                for pe in phase["pairs"]:
                    l1, l2 = pe["pair"]
                    rows = pe["rows"]
                    P = prod_pool.tile([128, len(rows), n_local], BF16, tag="P")
                    eng = nc.gpsimd if (l1, l2) in gp_pairs else nc.vector
                    for (m3, m1s, length, row0) in pe["prod_ops"]:
                        a0 = m1s + l1
                        # reversed f2: comp j = l2 - m2 = l2 - m3 + m1
                        j0 = l2 - m3 + m1s
                        eng.tensor_tensor(
                            P[:, row0 : row0 + length, :],
                            f1t[l1][:, a0 : a0 + length, :],
                            f2t[l2][:, j0 : j0 + length, :],
                            mybir.AluOpType.mult,
                        )
                    F = None
                    if pe["n_fold"]:
                        F = fold_pool.tile(
                            [128, pe["n_fold"], n_local], BF16, tag="F"
                        )
                        for (r0, ln, eps, f0, fh) in pe["fold_ops"]:
                            op = (
                                mybir.AluOpType.add
                                if eps > 0
                                else mybir.AluOpType.subtract
                            )
                            nc.vector.tensor_tensor(
                                F[:, f0 : f0 + fh, :],
                                P[:, r0 : r0 + fh, :],
                                P[:, r0 + ln - 1 : r0 + ln - 1 - fh : -1, :],
                                op,
                            )

                    def src(kind, row, sl=slice(None)):
                        t = P if kind == "prod" else F
                        return t[:, row, sl]

                    G = []
                    for members in pe["gsum_ops"]:
                        g = gsum_pool.tile([128, n_local], BF16, tag="G")
                        (k0, r0, s0), (k1, r1, s1) = members[0], members[1]
                        op = mybir.AluOpType.add if s1 > 0 else mybir.AluOpType.subtract
                        nc.vector.tensor_tensor(g[:], src(k0, r0), src(k1, r1), op)
                        for (k, r, sg) in members[2:]:
                            op = (
                                mybir.AluOpType.add
                                if sg > 0
                                else mybir.AluOpType.subtract
                            )
                            nc.vector.tensor_tensor(g[:], g[:], src(k, r), op)
                        G.append(g)

                    for (comp, vi, kind, ref) in pe["mms"]:
                        mm_done[comp] += 1
                        is_last = mm_done[comp] == mm_total[comp]
                        for h in range(n_halves):
                            sl = slice(h * mm_free, (h + 1) * mm_free)
                            rhs = G[ref][:, sl] if kind == "gsum" else src(kind, ref, sl)
                            nc.tensor.matmul(
                                psum[(comp, h)][:],
                                wvp[:, lay["local"][vi], :],
                                rhs,
                                start=False,
                                stop=is_last,
                                skip_group_check=True,
                            )
                        if is_last:
                            ci = comp_idx(*comp)
                            for h in range(n_halves):
                                sl = slice(h * mm_free, (h + 1) * mm_free)
                                st = stage_pool.tile([128, mm_free], F32, tag="stage")
                                nc.scalar.copy(st[:], psum[(comp, h)][:])
                                nc.sync.dma_start(out_d[:, ci, sl], st[:])

    nc.compile()
    return nc


_CACHE = {}


def _get_compiled():
    if "nc" not in _CACHE:
        tables = build_tables()
        _CACHE["tables"] = tables
        _CACHE["nc"] = build_nc(tables)
    return _CACHE["nc"], _CACHE["tables"]


def kernel(
    f1_l0, f1_l1, f1_l2, f1_l3,
    f2_l0, f2_l1, f2_l2, f2_l3,
    W_l0, W_l1, W_l2, W_l3,
    _trace=False,
):
    nc, tables = _get_compiled()

    f1_list = [np.asarray(f1_l0), np.asarray(f1_l1), np.asarray(f1_l2), np.asarray(f1_l3)]
    f2_list = [np.asarray(f2_l0), np.asarray(f2_l1), np.asarray(f2_l2), np.asarray(f2_l3)]
    W_list = [np.asarray(W_l0), np.asarray(W_l1), np.asarray(W_l2), np.asarray(W_l3)]

    def pack(fl, reverse_m=False):
        if reverse_m:
            fl = [f[:, ::-1, :] for f in fl]
        comps = np.concatenate(fl, axis=1)  # [N, 16, K] f32
        # -> [K, 16, N] bf16
        return np.ascontiguousarray(comps.transpose(2, 1, 0)).astype(ml_dtypes.bfloat16)

    F1 = pack(f1_list)
    F2 = pack(f2_list, reverse_m=True)
    WVfull = build_weight_variants(W_list, tables["variants"])  # [NV,128,128]
    wlayout, wtotal = phase_variant_layout(tables)
    WVp = np.zeros((wtotal, K, K), dtype=np.float32)
    for ph in PHASES:
        lay = wlayout[ph]
        for i, v in enumerate(lay["vids"]):
            WVp[lay["offset"] + i] = WVfull[v]
    WV = np.ascontiguousarray(
        WVp.astype(ml_dtypes.bfloat16).transpose(1, 0, 2).reshape(K, -1)
    )
    WID = np.ascontiguousarray(np.eye(K, dtype=np.float32).astype(ml_dtypes.bfloat16))

    in_maps = []
    for c in range(N_CORES):
        sl = slice(c * N_LOCAL, (c + 1) * N_LOCAL)
        in_maps.append(
            {
                "f1": np.ascontiguousarray(F1[:, :, sl]),
                "f2": np.ascontiguousarray(F2[:, :, sl]),
                "wv": WV,
                "wid": WID,
            }
        )

    res = run_bass_kernel_spmd(
        nc, in_maps, core_ids=list(range(N_CORES)), trace=_trace
    )
    _CACHE["last_result"] = res

    out_full = np.concatenate(
        [res.results[c]["out"] for c in range(N_CORES)], axis=2
    )  # [K, 16, N]
    outs = []
    offs = [0, 1, 4, 9, 16]
    for l in range(4):
        outs.append(
            np.ascontiguousarray(
                out_full[:, offs[l] : offs[l + 1], :].transpose(2, 1, 0)
            ).astype(np.float32)
        )
    return tuple(outs)


# revision 13
# speedup vs baseline: 1.1902x; 1.0278x over previous
"""Trainium2 Bass kernel for the CG tensor-product iteration (nn_CGIteration).

out[l3] = f1[l3] + concat_paths(einsum('abm,nak,nbk->nmk', C_p, f1[l1], f2[l2])) @ W[l3]

Self-contained: CG coefficients and the device schedule are computed here.
Data-parallel over nodes N: each of the 8 NeuronCores processes N/8 nodes.

Device algorithm (channel-major layout [k'=128 partitions, comp, n]):
  - products  P[m1,m2] = f1[l1,m1] * f2[l2,m2]   (DVE bf16 tensor_tensor,
    several (m1, m2-run) components per instruction via broadcast APs)
  - equal-|c| group sums S_g = sum_i sign_i P_i   (DVE add/sub)
  - PE matmuls psum[l3,m3] += (sign*|c| * W_path).T @ S_g with PSUM
    accumulation; the residual f1[l3] enters as an identity matmul
  - ACT evicts psum -> SBUF f32, DMA to DRAM
Weight variants (coeff * W_path, bf16) are prescaled on the host and DMAed.
"""

import sys
from math import factorial as fact

import numpy as np

if "/opt/trn_rl_repo" not in sys.path:  # harness safety; usually importable
    sys.path.append("/opt/trn_rl_repo")

import ml_dtypes

import concourse.mybir as mybir
import concourse.tile as tile
from concourse import bacc
from concourse.bass_utils import run_bass_kernel_spmd

BF16 = mybir.dt.bfloat16
F32 = mybir.dt.float32

L_MAX = 3
K = 128
N_TOTAL = 8192
N_CORES = 8
N_LOCAL = N_TOTAL // N_CORES

PATHS = [
    (l1, l2, l3)
    for l1 in range(L_MAX + 1)
    for l2 in range(L_MAX + 1)
    for l3 in range(abs(l1 - l2), min(l1 + l2, L_MAX) + 1)
]

PATH_BLOCK = {}
_counts = [0, 0, 0, 0]
for _p in PATHS:
    PATH_BLOCK[_p] = _counts[_p[2]]
    _counts[_p[2]] += 1

PHASE_BUCKETS = {
    "n32": (-3, -2),
    "n1": (-1,),
    "z0": (0,),
    "p1": (1,),
    "p32": (2, 3),
}
PHASES = ("n32", "n1", "z0", "p1", "p32")


def _phase_of(m3):
    for name, ms in PHASE_BUCKETS.items():
        if m3 in ms:
            return name
    raise ValueError(m3)


def _cg_coeff(l1, m1, l2, m2, l3, m3):
    if m1 + m2 != m3:
        return 0.0
    pref = (
        (2 * l3 + 1)
        * fact(l3 + l1 - l2)
        * fact(l3 - l1 + l2)
        * fact(l1 + l2 - l3)
        / fact(l1 + l2 + l3 + 1)
    ) ** 0.5
    pref *= (
        fact(l3 + m3)
        * fact(l3 - m3)
        * fact(l1 - m1)
        * fact(l1 + m1)
        * fact(l2 - m2)
        * fact(l2 + m2)
    ) ** 0.5
    s = 0.0
    for k in range(
        max(0, l2 - l3 - m1, l1 - l3 + m2),
        min(l1 + l2 - l3, l1 - m1, l2 + m2) + 1,
    ):
        s += (-1) ** k / (
            fact(k)
            * fact(l1 + l2 - l3 - k)
            * fact(l1 - m1 - k)
            * fact(l2 + m2 - k)
            * fact(l3 - l2 + m1 + k)
            * fact(l3 - l1 - m2 + k)
        )
    return pref * s


def comp_idx(l, m):
    return l * l + (m + l)


def build_tables(coeff_tol=1e-9):
    band = {}
    for p in PATHS:
        l1, l2, l3 = p
        for m3 in range(-l3, l3 + 1):
            terms = []
            for m1 in range(-l1, l1 + 1):
                m2 = m3 - m1
                if -l2 <= m2 <= l2:
                    c = _cg_coeff(l1, m1, l2, m2, l3, m3)
                    if abs(c) > coeff_tol:
                        terms.append((m1, m2, c))
            if terms:
                band[(p, m3)] = terms

    variants = [("identity", 1.0)]
    variant_idx = {("identity", 1.0): 0}

    def get_variant(p, coeff):
        key = (p, round(coeff, 9))
        if key not in variant_idx:
            variant_idx[key] = len(variants)
            variants.append(key)
        return variant_idx[key]

    pairs = sorted(
        {(p[0], p[1]) for p in PATHS},
        key=lambda pr: (max(2 * pr[0], 2 * pr[1] + 1), pr),
    )
    phases = {}
    for ph in PHASES:
        comps = []
        for l3 in range(L_MAX + 1):
            for m3 in range(-l3, l3 + 1):
                if _phase_of(m3) == ph and any(
                    (p, m3) in band for p in PATHS if p[2] == l3
                ):
                    comps.append((l3, m3))
        pair_entries = []
        comp_matmuls = {c: [] for c in comps}
        for pr in pairs:
            l1, l2 = pr
            pr_paths = [p for p in PATHS if (p[0], p[1]) == pr]
            used = set()
            for p in pr_paths:
                for m3 in range(-p[2], p[2] + 1):
                    if _phase_of(m3) != ph:
                        continue
                    for (m1, m2, c) in band.get((p, m3), []):
                        used.add((m1, m2))
            if not used:
                continue
            # zip products: for fixed m3, (m1, m3-m1) pairs align f1 comps
            # (ascending m1) with reversed-order f2 comps (ascending index).
            row_of = {}
            rows = []
            prod_ops = []  # (m3, m1_start, length, row_start)
            for m3 in sorted({m1 + m2 for (m1, m2) in used}):
                m1s = sorted(m1 for (m1, m2) in used if m1 + m2 == m3)
                m1_start, m1_end = m1s[0], m1s[-1]
                row_start = len(rows)
                for m1 in range(m1_start, m1_end + 1):
                    row_of[(m1, m3 - m1)] = len(rows)
                    rows.append((m1, m3 - m1))
                prod_ops.append((m3, m1_start, m1_end - m1_start + 1, row_start))
            # mirror folds (l1 == l2): within a zip, c(m3-m1) = (-1)^l3 c(m1),
            # so rows i and L-1-i combine as row_i +/- row_{L-1-i} for all
            # bands of one l3-parity -- one DVE op per (zip, parity).
            zip_info = {m3: (m1s, ln, r0) for (m3, m1s, ln, r0) in prod_ops}
            fold_ops = []  # (row0, L, eps, frow0, h)
            fold_index = {}
            n_fold = 0
            if l1 == l2:
                need = set()
                for p in pr_paths:
                    l3 = p[2]
                    eps = 1 if l3 % 2 == 0 else -1
                    for m3 in range(-l3, l3 + 1):
                        if _phase_of(m3) != ph or (p, m3) not in band:
                            continue
                        m1s, ln, r0 = zip_info[m3]
                        h = ln // 2
                        for (m1, m2, c) in band[(p, m3)]:
                            i = m1 - m1s
                            if i < h:
                                cm = _cg_coeff(l1, m2, l2, m1, l3, m3)
                                assert abs(cm - eps * c) < 1e-9
                                need.add((m3, eps))
                for (m3, eps) in sorted(need, key=lambda t: (t[0], -t[1])):
                    m1s, ln, r0 = zip_info[m3]
                    h = ln // 2
                    fold_ops.append((r0, ln, eps, n_fold, h))
                    for i in range(h):
                        fold_index[(m3, eps, i)] = n_fold + i
                    n_fold += h

            gsum_ops = []  # members: (kind, row, relsign)
            pair_mms = []
            for p in pr_paths:
                l3 = p[2]
                eps = 1 if l3 % 2 == 0 else -1
                for m3 in range(-l3, l3 + 1):
                    if _phase_of(m3) != ph or (p, m3) not in band:
                        continue
                    terms = band[(p, m3)]
                    # reduce via folds
                    red = []  # (kind, row, c)
                    if l1 == l2:
                        m1s, ln, r0 = zip_info[m3]
                        h = ln // 2
                        for (m1, m2, c) in terms:
                            i = m1 - m1s
                            if i < h:
                                red.append(("fold", fold_index[(m3, eps, i)], c))
                            elif i == ln - 1 - i:
                                red.append(("prod", row_of[(m1, m2)], c))
                            # i > mirror: covered by fold
                    else:
                        red = [("prod", row_of[(m1, m2)], c) for (m1, m2, c) in terms]
                    gs = {}
                    for (kind, row, c) in red:
                        gs.setdefault(round(abs(c), 9), []).append(
                            (kind, row, 1.0 if c > 0 else -1.0)
                        )
                    for gamma, members in sorted(gs.items()):
                        sigma1 = members[0][2]
                        vi = get_variant(p, sigma1 * gamma)
                        if len(members) == 1:
                            kind, row, _ = members[0]
                            pair_mms.append(((l3, m3), vi, kind, row))
                        else:
                            gid = len(gsum_ops)
                            gsum_ops.append(
                                [(k, r, sg * sigma1) for (k, r, sg) in members]
                            )
                            pair_mms.append(((l3, m3), vi, "gsum", gid))
            pair_entries.append(
                dict(pair=pr, rows=rows, prod_ops=prod_ops, fold_ops=fold_ops,
                     n_fold=n_fold, gsum_ops=gsum_ops, mms=pair_mms)
            )
            for mm in pair_mms:
                comp_matmuls[mm[0]].append((len(pair_entries) - 1,) + mm[1:])
        phases[ph] = dict(comps=comps, pairs=pair_entries, comp_matmuls=comp_matmuls)

    return dict(variants=variants, phases=phases)


def build_weight_variants(W_list, variants):
    out = np.zeros((len(variants), K, K), dtype=np.float32)
    for i, (p, coeff) in enumerate(variants):
        if p == "identity":
            out[i] = np.eye(K, dtype=np.float32)
        else:
            b = PATH_BLOCK[p]
            out[i] = coeff * W_list[p[2]][b * K : (b + 1) * K, :]
    return out



def phase_variant_layout(tables):
    """Per-(phase, pair) variant lists (global ids) + dram offsets."""
    layout = {}
    off = 0
    for ph in PHASES:
        phase = tables["phases"][ph]
        for pi, pe in enumerate(phase["pairs"]):
            vids = sorted({mm[1] for mm in pe["mms"]})
            local = {v: i for i, v in enumerate(vids)}
            layout[(ph, pi)] = dict(vids=vids, local=local, offset=off)
            off += len(vids)
    return layout, off


def build_nc(tables, n_local=N_LOCAL, mm_free=512, gp_pairs=()):
    wlayout, wtotal = phase_variant_layout(tables)
    nc = bacc.Bacc("TRN2", target_bir_lowering=False, debug=False)

    f1_d = nc.dram_tensor("f1", [128, 16, n_local], BF16, kind="ExternalInput")
    f2_d = nc.dram_tensor("f2", [128, 16, n_local], BF16, kind="ExternalInput")
    wid_d = nc.dram_tensor("wid", [128, 128], BF16, kind="ExternalInput")
    wv_d = nc.dram_tensor("wv", [128, wtotal * 128], BF16, kind="ExternalInput")
    out_d = nc.dram_tensor("out", [128, 16, n_local], F32, kind="ExternalOutput")

    n_halves = n_local // mm_free

    with tile.TileContext(nc) as tc:
        with (
            tc.tile_pool(name="inp", bufs=1) as inp_pool,
            tc.tile_pool(name="wpool", bufs=4) as w_pool,
            tc.tile_pool(name="widp", bufs=1) as wid_pool,
            tc.tile_pool(name="prod", bufs=4) as prod_pool,
            tc.tile_pool(name="fold", bufs=2) as fold_pool,
            tc.tile_pool(name="gsum", bufs=6) as gsum_pool,
            tc.tile_pool(name="stage", bufs=4) as stage_pool,
            tc.tile_pool(name="psum", bufs=8, space="PSUM") as psum_pool,
        ):
            wid = wid_pool.tile([128, 128], BF16, tag="wid")
            nc.sync.dma_start(wid[:], wid_d[:])
            f1t = []
            f2t = []
            for l in range(4):
                nm = 2 * l + 1
                t1 = inp_pool.tile([128, nm, n_local], BF16, tag=f"f1_{l}",
                                   name=f"f1_{l}")
                t2 = inp_pool.tile([128, nm, n_local], BF16, tag=f"f2_{l}",
                                   name=f"f2_{l}")
                nc.sync.dma_start(t1[:], f1_d[:, l * l : l * l + nm, :])
                nc.sync.dma_start(t2[:], f2_d[:, l * l : l * l + nm, :])
                f1t.append(t1)
                f2t.append(t2)

            for ph in PHASES:
                phase = tables["phases"][ph]
                comps = phase["comps"]
                mm_total = {c: len(phase["comp_matmuls"][c]) for c in comps}
                mm_done = {c: 0 for c in comps}

                psum = {}
                for comp in comps:
                    for h in range(n_halves):
                        psum[(comp, h)] = psum_pool.tile(
                            [128, mm_free],
                            F32,
                            tag="ps",
                            name=f"ps_{ph}_{comp_idx(*comp)}_{h}",
                        )
                for comp in comps:
                    l3, m3 = comp
                    for h in range(n_halves):
                        sl = slice(h * mm_free, (h + 1) * mm_free)
                        nc.tensor.matmul(
                            psum[(comp, h)][:],
                            wid[:],
                            f1t[l3][:, m3 + l3, sl],
                            start=True,
                            stop=False,
                            skip_group_check=True,
                        )

                for pi, pe in enumerate(phase["pairs"]):
                    l1, l2 = pe["pair"]
                    rows = pe["rows"]
                    lay = wlayout[(ph, pi)]
                    nvp = len(lay["vids"])
                    wvp = w_pool.tile([128, nvp, 128], BF16, tag="wvp",
                                      name=f"wvp_{ph}_{pi}")
                    nc.sync.dma_start(
                        wvp[:],
                        wv_d[:, lay["offset"] * 128 : (lay["offset"] + nvp) * 128]
                        .rearrange("p (v k) -> p v k", v=nvp),
                    )
                    P = prod_pool.tile([128, len(rows), n_local], BF16, tag="P")
                    eng = nc.gpsimd if (l1, l2) in gp_pairs else nc.vector
                    for (m3, m1s, length, row0) in pe["prod_ops"]:
                        a0 = m1s + l1
                        # reversed f2: comp j = l2 - m2 = l2 - m3 + m1
                        j0 = l2 - m3 + m1s
                        eng.tensor_tensor(
                            P[:, row0 : row0 + length, :],
                            f1t[l1][:, a0 : a0 + length, :],
                            f2t[l2][:, j0 : j0 + length, :],
                            mybir.AluOpType.mult,
                        )
                    F = None
                    if pe["n_fold"]:
                        F = fold_pool.tile(
                            [128, pe["n_fold"], n_local], BF16, tag="F"
                        )
                        for (r0, ln, eps, f0, fh) in pe["fold_ops"]:
                            op = (
                                mybir.AluOpType.add
                                if eps > 0
                                else mybir.AluOpType.subtract
                            )
                            nc.vector.tensor_tensor(
                                F[:, f0 : f0 + fh, :],
                                P[:, r0 : r0 + fh, :],
                                P[:, r0 + ln - 1 : r0 + ln - 1 - fh : -1, :],
                                op,
                            )

                    def src(kind, row, sl=slice(None)):
                        t = P if kind == "prod" else F
                        return t[:, row, sl]

                    G = []
                    for members in pe["gsum_ops"]:
                        g = gsum_pool.tile([128, n_local], BF16, tag="G")
                        (k0, r0, s0), (k1, r1, s1) = members[0], members[1]
                        op = mybir.AluOpType.add if s1 > 0 else mybir.AluOpType.subtract
                        nc.vector.tensor_tensor(g[:], src(k0, r0), src(k1, r1), op)
                        for (k, r, sg) in members[2:]:
                            op = (
                                mybir.AluOpType.add
                                if sg > 0
                                else mybir.AluOpType.subtract
                            )
                            nc.vector.tensor_tensor(g[:], g[:], src(k, r), op)
                        G.append(g)

                    for (comp, vi, kind, ref) in pe["mms"]:
                        mm_done[comp] += 1
                        is_last = mm_done[comp] == mm_total[comp]
                        for h in range(n_halves):
                            sl = slice(h * mm_free, (h + 1) * mm_free)
                            rhs = G[ref][:, sl] if kind == "gsum" else src(kind, ref, sl)
                            nc.tensor.matmul(
                                psum[(comp, h)][:],
                                wvp[:, lay["local"][vi], :],
                                rhs,
                                start=False,
                                stop=is_last,
                                skip_group_check=True,
                            )
                        if is_last:
                            ci = comp_idx(*comp)
                            for h in range(n_halves):
                                sl = slice(h * mm_free, (h + 1) * mm_free)
                                st = stage_pool.tile([128, mm_free], F32, tag="stage")
                                nc.scalar.copy(st[:], psum[(comp, h)][:])
                                nc.sync.dma_start(out_d[:, ci, sl], st[:])

    nc.compile()
    return nc


_CACHE = {}


def _get_compiled():
    if "nc" not in _CACHE:
        tables = build_tables()
        _CACHE["tables"] = tables
        _CACHE["nc"] = build_nc(tables)
    return _CACHE["nc"], _CACHE["tables"]


def kernel(
    f1_l0, f1_l1, f1_l2, f1_l3,
    f2_l0, f2_l1, f2_l2, f2_l3,
    W_l0, W_l1, W_l2, W_l3,
    _trace=False,
):
    nc, tables = _get_compiled()

    f1_list = [np.asarray(f1_l0), np.asarray(f1_l1), np.asarray(f1_l2), np.asarray(f1_l3)]
    f2_list = [np.asarray(f2_l0), np.asarray(f2_l1), np.asarray(f2_l2), np.asarray(f2_l3)]
    W_list = [np.asarray(W_l0), np.asarray(W_l1), np.asarray(W_l2), np.asarray(W_l3)]

    def pack(fl, reverse_m=False):
        if reverse_m:
            fl = [f[:, ::-1, :] for f in fl]
        comps = np.concatenate(fl, axis=1)  # [N, 16, K] f32
        # -> [K, 16, N] bf16
        return np.ascontiguousarray(comps.transpose(2, 1, 0)).astype(ml_dtypes.bfloat16)

    F1 = pack(f1_list)
    F2 = pack(f2_list, reverse_m=True)
    WVfull = build_weight_variants(W_list, tables["variants"])  # [NV,128,128]
    wlayout, wtotal = phase_variant_layout(tables)
    WVp = np.zeros((wtotal, K, K), dtype=np.float32)
    for key, lay in wlayout.items():
        for i, v in enumerate(lay["vids"]):
            WVp[lay["offset"] + i] = WVfull[v]
    WV = np.ascontiguousarray(
        WVp.astype(ml_dtypes.bfloat16).transpose(1, 0, 2).reshape(K, -1)
    )
    WID = np.ascontiguousarray(np.eye(K, dtype=np.float32).astype(ml_dtypes.bfloat16))

    in_maps = []
    for c in range(N_CORES):
        sl = slice(c * N_LOCAL, (c + 1) * N_LOCAL)
        in_maps.append(
            {
                "f1": np.ascontiguousarray(F1[:, :, sl]),
                "f2": np.ascontiguousarray(F2[:, :, sl]),
                "wv": WV,
                "wid": WID,
            }
        )

    res = run_bass_kernel_spmd(
        nc, in_maps, core_ids=list(range(N_CORES)), trace=_trace
    )
    _CACHE["last_result"] = res

    out_full = np.concatenate(
        [res.results[c]["out"] for c in range(N_CORES)], axis=2
    )  # [K, 16, N]
    outs = []
    offs = [0, 1, 4, 9, 16]
    for l in range(4):
        outs.append(
            np.ascontiguousarray(
                out_full[:, offs[l] : offs[l + 1], :].transpose(2, 1, 0)
            ).astype(np.float32)
        )
    return tuple(outs)


# revision 14
# speedup vs baseline: 1.2539x; 1.0535x over previous
"""Trainium2 Bass kernel for the CG tensor-product iteration (nn_CGIteration).

out[l3] = f1[l3] + concat_paths(einsum('abm,nak,nbk->nmk', C_p, f1[l1], f2[l2])) @ W[l3]

Self-contained: CG coefficients and the device schedule are computed here.
Data-parallel over nodes N: each of the 8 NeuronCores processes N/8 nodes.

Device algorithm (channel-major layout [k'=128 partitions, comp, n]):
  - products  P[m1,m2] = f1[l1,m1] * f2[l2,m2]   (DVE bf16 tensor_tensor,
    several (m1, m2-run) components per instruction via broadcast APs)
  - equal-|c| group sums S_g = sum_i sign_i P_i   (DVE add/sub)
  - PE matmuls psum[l3,m3] += (sign*|c| * W_path).T @ S_g with PSUM
    accumulation; the residual f1[l3] enters as an identity matmul
  - ACT evicts psum -> SBUF f32, DMA to DRAM
Weight variants (coeff * W_path, bf16) are prescaled on the host and DMAed.
"""

import sys
from math import factorial as fact

import numpy as np

if "/opt/trn_rl_repo" not in sys.path:  # harness safety; usually importable
    sys.path.append("/opt/trn_rl_repo")

import ml_dtypes

import concourse.mybir as mybir
import concourse.tile as tile
from concourse import bacc
from concourse.bass_utils import run_bass_kernel_spmd

BF16 = mybir.dt.bfloat16
F32 = mybir.dt.float32

L_MAX = 3
K = 128
N_TOTAL = 8192
N_CORES = 8
N_LOCAL = N_TOTAL // N_CORES

PATHS = [
    (l1, l2, l3)
    for l1 in range(L_MAX + 1)
    for l2 in range(L_MAX + 1)
    for l3 in range(abs(l1 - l2), min(l1 + l2, L_MAX) + 1)
]

PATH_BLOCK = {}
_counts = [0, 0, 0, 0]
for _p in PATHS:
    PATH_BLOCK[_p] = _counts[_p[2]]
    _counts[_p[2]] += 1

PHASE_BUCKETS = {
    "n32": (-3, -2),
    "n1": (-1,),
    "z0": (0,),
    "p1": (1,),
    "p32": (2, 3),
}
PHASES = ("n32", "n1", "z0", "p1", "p32")


def _phase_of(m3):
    for name, ms in PHASE_BUCKETS.items():
        if m3 in ms:
            return name
    raise ValueError(m3)


def _cg_coeff(l1, m1, l2, m2, l3, m3):
    if m1 + m2 != m3:
        return 0.0
    pref = (
        (2 * l3 + 1)
        * fact(l3 + l1 - l2)
        * fact(l3 - l1 + l2)
        * fact(l1 + l2 - l3)
        / fact(l1 + l2 + l3 + 1)
    ) ** 0.5
    pref *= (
        fact(l3 + m3)
        * fact(l3 - m3)
        * fact(l1 - m1)
        * fact(l1 + m1)
        * fact(l2 - m2)
        * fact(l2 + m2)
    ) ** 0.5
    s = 0.0
    for k in range(
        max(0, l2 - l3 - m1, l1 - l3 + m2),
        min(l1 + l2 - l3, l1 - m1, l2 + m2) + 1,
    ):
        s += (-1) ** k / (
            fact(k)
            * fact(l1 + l2 - l3 - k)
            * fact(l1 - m1 - k)
            * fact(l2 + m2 - k)
            * fact(l3 - l2 + m1 + k)
            * fact(l3 - l1 - m2 + k)
        )
    return pref * s


def comp_idx(l, m):
    return l * l + (m + l)


def build_tables(coeff_tol=1e-9, cvt_every=2):
    _cvt = [0, cvt_every]
    band = {}
    for p in PATHS:
        l1, l2, l3 = p
        for m3 in range(-l3, l3 + 1):
            terms = []
            for m1 in range(-l1, l1 + 1):
                m2 = m3 - m1
                if -l2 <= m2 <= l2:
                    c = _cg_coeff(l1, m1, l2, m2, l3, m3)
                    if abs(c) > coeff_tol:
                        terms.append((m1, m2, c))
            if terms:
                band[(p, m3)] = terms

    variants = [("identity", 1.0)]
    variant_idx = {("identity", 1.0): 0}

    def get_variant(p, coeff):
        key = (p, round(coeff, 9))
        if key not in variant_idx:
            variant_idx[key] = len(variants)
            variants.append(key)
        return variant_idx[key]

    pairs = sorted(
        {(p[0], p[1]) for p in PATHS},
        key=lambda pr: (max(2 * pr[0], 2 * pr[1] + 1), pr),
    )
    phases = {}
    for ph in PHASES:
        comps = []
        for l3 in range(L_MAX + 1):
            for m3 in range(-l3, l3 + 1):
                if _phase_of(m3) == ph and any(
                    (p, m3) in band for p in PATHS if p[2] == l3
                ):
                    comps.append((l3, m3))
        pair_entries = []
        comp_matmuls = {c: [] for c in comps}
        for pr in pairs:
            l1, l2 = pr
            pr_paths = [p for p in PATHS if (p[0], p[1]) == pr]
            used = set()
            for p in pr_paths:
                for m3 in range(-p[2], p[2] + 1):
                    if _phase_of(m3) != ph:
                        continue
                    for (m1, m2, c) in band.get((p, m3), []):
                        used.add((m1, m2))
            if not used:
                continue
            # zip products: for fixed m3, (m1, m3-m1) pairs align f1 comps
            # (ascending m1) with reversed-order f2 comps (ascending index).
            row_of = {}
            rows = []
            prod_ops = []  # (m3, m1_start, length, row_start)
            for m3 in sorted({m1 + m2 for (m1, m2) in used}):
                m1s = sorted(m1 for (m1, m2) in used if m1 + m2 == m3)
                m1_start, m1_end = m1s[0], m1s[-1]
                row_start = len(rows)
                for m1 in range(m1_start, m1_end + 1):
                    row_of[(m1, m3 - m1)] = len(rows)
                    rows.append((m1, m3 - m1))
                prod_ops.append((m3, m1_start, m1_end - m1_start + 1, row_start))
            # mirror folds (l1 == l2): within a zip, c(m3-m1) = (-1)^l3 c(m1),
            # so rows i and L-1-i combine as row_i +/- row_{L-1-i} for all
            # bands of one l3-parity -- one DVE op per (zip, parity).
            zip_info = {m3: (m1s, ln, r0) for (m3, m1s, ln, r0) in prod_ops}
            fold_ops = []  # (row0, L, eps, frow0, h)
            fold_index = {}
            n_fold = 0
            if l1 == l2:
                need = set()
                for p in pr_paths:
                    l3 = p[2]
                    eps = 1 if l3 % 2 == 0 else -1
                    for m3 in range(-l3, l3 + 1):
                        if _phase_of(m3) != ph or (p, m3) not in band:
                            continue
                        m1s, ln, r0 = zip_info[m3]
                        h = ln // 2
                        for (m1, m2, c) in band[(p, m3)]:
                            i = m1 - m1s
                            if i < h:
                                cm = _cg_coeff(l1, m2, l2, m1, l3, m3)
                                assert abs(cm - eps * c) < 1e-9
                                need.add((m3, eps))
                for (m3, eps) in sorted(need, key=lambda t: (t[0], -t[1])):
                    m1s, ln, r0 = zip_info[m3]
                    h = ln // 2
                    fold_ops.append((r0, ln, eps, n_fold, h))
                    for i in range(h):
                        fold_index[(m3, eps, i)] = n_fold + i
                    n_fold += h

            gsum_ops = []  # members: (kind, row, relsign)
            pair_mms = []
            for p in pr_paths:
                l3 = p[2]
                eps = 1 if l3 % 2 == 0 else -1
                for m3 in range(-l3, l3 + 1):
                    if _phase_of(m3) != ph or (p, m3) not in band:
                        continue
                    terms = band[(p, m3)]
                    # reduce via folds
                    red = []  # (kind, row, c)
                    if l1 == l2:
                        m1s, ln, r0 = zip_info[m3]
                        h = ln // 2
                        for (m1, m2, c) in terms:
                            i = m1 - m1s
                            if i < h:
                                red.append(("fold", fold_index[(m3, eps, i)], c))
                            elif i == ln - 1 - i:
                                red.append(("prod", row_of[(m1, m2)], c))
                            # i > mirror: covered by fold
                    else:
                        red = [("prod", row_of[(m1, m2)], c) for (m1, m2, c) in terms]
                    gs = {}
                    for (kind, row, c) in red:
                        gs.setdefault(round(abs(c), 9), []).append(
                            (kind, row, 1.0 if c > 0 else -1.0)
                        )
                    for gamma, members in sorted(gs.items()):
                        sigma1 = members[0][2]
                        if len(members) == 1:
                            kind, row, _ = members[0]
                            vi = get_variant(p, sigma1 * gamma)
                            pair_mms.append(((l3, m3), vi, kind, row))
                        elif len(members) == 2 and (_cvt[0] % _cvt[1] == 0):
                            # direct fold to PE: one matmul per member
                            _cvt[0] += 1
                            for (k, r, sg) in members:
                                vim = get_variant(p, sg * gamma)
                                pair_mms.append(((l3, m3), vim, k, r))
                        else:
                            if len(members) == 2:
                                _cvt[0] += 1
                            vi = get_variant(p, sigma1 * gamma)
                            gid = len(gsum_ops)
                            gsum_ops.append(
                                [(k, r, sg * sigma1) for (k, r, sg) in members]
                            )
                            pair_mms.append(((l3, m3), vi, "gsum", gid))
            pair_entries.append(
                dict(pair=pr, rows=rows, prod_ops=prod_ops, fold_ops=fold_ops,
                     n_fold=n_fold, gsum_ops=gsum_ops, mms=pair_mms)
            )
            for mm in pair_mms:
                comp_matmuls[mm[0]].append((len(pair_entries) - 1,) + mm[1:])
        phases[ph] = dict(comps=comps, pairs=pair_entries, comp_matmuls=comp_matmuls)

    return dict(variants=variants, phases=phases)


def build_weight_variants(W_list, variants):
    out = np.zeros((len(variants), K, K), dtype=np.float32)
    for i, (p, coeff) in enumerate(variants):
        if p == "identity":
            out[i] = np.eye(K, dtype=np.float32)
        else:
            b = PATH_BLOCK[p]
            out[i] = coeff * W_list[p[2]][b * K : (b + 1) * K, :]
    return out



def phase_variant_layout(tables):
    """Per-(phase, pair) variant lists (global ids) + dram offsets."""
    layout = {}
    off = 0
    for ph in PHASES:
        phase = tables["phases"][ph]
        for pi, pe in enumerate(phase["pairs"]):
            vids = sorted({mm[1] for mm in pe["mms"]})
            local = {v: i for i, v in enumerate(vids)}
            layout[(ph, pi)] = dict(vids=vids, local=local, offset=off)
            off += len(vids)
    return layout, off


def build_nc(tables, n_local=N_LOCAL, mm_free=512, gp_pairs=()):
    wlayout, wtotal = phase_variant_layout(tables)
    nc = bacc.Bacc("TRN2", target_bir_lowering=False, debug=False)

    f1_d = nc.dram_tensor("f1", [128, 16, n_local], BF16, kind="ExternalInput")
    f2_d = nc.dram_tensor("f2", [128, 16, n_local], BF16, kind="ExternalInput")
    wid_d = nc.dram_tensor("wid", [128, 128], BF16, kind="ExternalInput")
    wv_d = nc.dram_tensor("wv", [128, wtotal * 128], BF16, kind="ExternalInput")
    out_d = nc.dram_tensor("out", [128, 16, n_local], F32, kind="ExternalOutput")

    n_halves = n_local // mm_free

    with tile.TileContext(nc) as tc:
        with (
            tc.tile_pool(name="inp", bufs=1) as inp_pool,
            tc.tile_pool(name="wpool", bufs=4) as w_pool,
            tc.tile_pool(name="widp", bufs=1) as wid_pool,
            tc.tile_pool(name="prod", bufs=4) as prod_pool,
            tc.tile_pool(name="fold", bufs=2) as fold_pool,
            tc.tile_pool(name="gsum", bufs=6) as gsum_pool,
            tc.tile_pool(name="stage", bufs=4) as stage_pool,
            tc.tile_pool(name="psum", bufs=8, space="PSUM") as psum_pool,
        ):
            wid = wid_pool.tile([128, 128], BF16, tag="wid")
            nc.sync.dma_start(wid[:], wid_d[:])
            f1t = []
            f2t = []
            for l in range(4):
                nm = 2 * l + 1
                t1 = inp_pool.tile([128, nm, n_local], BF16, tag=f"f1_{l}",
                                   name=f"f1_{l}")
                t2 = inp_pool.tile([128, nm, n_local], BF16, tag=f"f2_{l}",
                                   name=f"f2_{l}")
                hn = n_local // 2
                nc.sync.dma_start(t1[:, :, 0:hn], f1_d[:, l * l : l * l + nm, 0:hn])
                nc.scalar.dma_start(
                    t1[:, :, hn:], f1_d[:, l * l : l * l + nm, hn:]
                )
                nc.sync.dma_start(t2[:, :, 0:hn], f2_d[:, l * l : l * l + nm, 0:hn])
                nc.scalar.dma_start(
                    t2[:, :, hn:], f2_d[:, l * l : l * l + nm, hn:]
                )
                f1t.append(t1)
                f2t.append(t2)

            for ph in PHASES:
                phase = tables["phases"][ph]
                comps = phase["comps"]
                mm_total = {c: len(phase["comp_matmuls"][c]) for c in comps}
                mm_done = {c: 0 for c in comps}

                psum = {}
                for comp in comps:
                    for h in range(n_halves):
                        psum[(comp, h)] = psum_pool.tile(
                            [128, mm_free],
                            F32,
                            tag="ps",
                            name=f"ps_{ph}_{comp_idx(*comp)}_{h}",
                        )
                for comp in comps:
                    l3, m3 = comp
                    for h in range(n_halves):
                        sl = slice(h * mm_free, (h + 1) * mm_free)
                        nc.tensor.matmul(
                            psum[(comp, h)][:],
                            wid[:],
                            f1t[l3][:, m3 + l3, sl],
                            start=True,
                            stop=False,
                            skip_group_check=True,
                        )

                for pi, pe in enumerate(phase["pairs"]):
                    l1, l2 = pe["pair"]
                    rows = pe["rows"]
                    lay = wlayout[(ph, pi)]
                    nvp = len(lay["vids"])
                    wvp = w_pool.tile([128, nvp, 128], BF16, tag="wvp",
                                      name=f"wvp_{ph}_{pi}")
                    nc.sync.dma_start(
                        wvp[:],
                        wv_d[:, lay["offset"] * 128 : (lay["offset"] + nvp) * 128]
                        .rearrange("p (v k) -> p v k", v=nvp),
                    )
                    P = prod_pool.tile([128, len(rows), n_local], BF16, tag="P")
                    eng = nc.gpsimd if (l1, l2) in gp_pairs else nc.vector
                    for (m3, m1s, length, row0) in pe["prod_ops"]:
                        a0 = m1s + l1
                        # reversed f2: comp j = l2 - m2 = l2 - m3 + m1
                        j0 = l2 - m3 + m1s
                        eng.tensor_tensor(
                            P[:, row0 : row0 + length, :],
                            f1t[l1][:, a0 : a0 + length, :],
                            f2t[l2][:, j0 : j0 + length, :],
                            mybir.AluOpType.mult,
                        )
                    F = None
                    if pe["n_fold"]:
                        F = fold_pool.tile(
                            [128, pe["n_fold"], n_local], BF16, tag="F"
                        )
                        for (r0, ln, eps, f0, fh) in pe["fold_ops"]:
                            op = (
                                mybir.AluOpType.add
                                if eps > 0
                                else mybir.AluOpType.subtract
                            )
                            nc.vector.tensor_tensor(
                                F[:, f0 : f0 + fh, :],
                                P[:, r0 : r0 + fh, :],
                                P[:, r0 + ln - 1 : r0 + ln - 1 - fh : -1, :],
                                op,
                            )

                    def src(kind, row, sl=slice(None)):
                        t = P if kind == "prod" else F
                        return t[:, row, sl]

                    G = []
                    for members in pe["gsum_ops"]:
                        g = gsum_pool.tile([128, n_local], BF16, tag="G")
                        (k0, r0, s0), (k1, r1, s1) = members[0], members[1]
                        op = mybir.AluOpType.add if s1 > 0 else mybir.AluOpType.subtract
                        nc.vector.tensor_tensor(g[:], src(k0, r0), src(k1, r1), op)
                        for (k, r, sg) in members[2:]:
                            op = (
                                mybir.AluOpType.add
                                if sg > 0
                                else mybir.AluOpType.subtract
                            )
                            nc.vector.tensor_tensor(g[:], g[:], src(k, r), op)
                        G.append(g)

                    for (comp, vi, kind, ref) in pe["mms"]:
                        mm_done[comp] += 1
                        is_last = mm_done[comp] == mm_total[comp]
                        for h in range(n_halves):
                            sl = slice(h * mm_free, (h + 1) * mm_free)
                            rhs = G[ref][:, sl] if kind == "gsum" else src(kind, ref, sl)
                            nc.tensor.matmul(
                                psum[(comp, h)][:],
                                wvp[:, lay["local"][vi], :],
                                rhs,
                                start=False,
                                stop=is_last,
                                skip_group_check=True,
                            )
                        if is_last:
                            ci = comp_idx(*comp)
                            for h in range(n_halves):
                                sl = slice(h * mm_free, (h + 1) * mm_free)
                                st = stage_pool.tile([128, mm_free], F32, tag="stage")
                                nc.scalar.copy(st[:], psum[(comp, h)][:])
                                nc.sync.dma_start(out_d[:, ci, sl], st[:])

    nc.compile()
    return nc


_CACHE = {}


def _get_compiled():
    if "nc" not in _CACHE:
        tables = build_tables()
        _CACHE["tables"] = tables
        _CACHE["nc"] = build_nc(tables)
    return _CACHE["nc"], _CACHE["tables"]


def kernel(
    f1_l0, f1_l1, f1_l2, f1_l3,
    f2_l0, f2_l1, f2_l2, f2_l3,
    W_l0, W_l1, W_l2, W_l3,
    _trace=False,
):
    nc, tables = _get_compiled()

    f1_list = [np.asarray(f1_l0), np.asarray(f1_l1), np.asarray(f1_l2), np.asarray(f1_l3)]
    f2_list = [np.asarray(f2_l0), np.asarray(f2_l1), np.asarray(f2_l2), np.asarray(f2_l3)]
    W_list = [np.asarray(W_l0), np.asarray(W_l1), np.asarray(W_l2), np.asarray(W_l3)]

    def pack(fl, reverse_m=False):
        if reverse_m:
            fl = [f[:, ::-1, :] for f in fl]
        comps = np.concatenate(fl, axis=1)  # [N, 16, K] f32
        # -> [K, 16, N] bf16
        return np.ascontiguousarray(comps.transpose(2, 1, 0)).astype(ml_dtypes.bfloat16)

    F1 = pack(f1_list)
    F2 = pack(f2_list, reverse_m=True)
    WVfull = build_weight_variants(W_list, tables["variants"])  # [NV,128,128]
    wlayout, wtotal = phase_variant_layout(tables)
    WVp = np.zeros((wtotal, K, K), dtype=np.float32)
    for key, lay in wlayout.items():
        for i, v in enumerate(lay["vids"]):
            WVp[lay["offset"] + i] = WVfull[v]
    WV = np.ascontiguousarray(
        WVp.astype(ml_dtypes.bfloat16).transpose(1, 0, 2).reshape(K, -1)
    )
    WID = np.ascontiguousarray(np.eye(K, dtype=np.float32).astype(ml_dtypes.bfloat16))

    in_maps = []
    for c in range(N_CORES):
        sl = slice(c * N_LOCAL, (c + 1) * N_LOCAL)
        in_maps.append(
            {
                "f1": np.ascontiguousarray(F1[:, :, sl]),
                "f2": np.ascontiguousarray(F2[:, :, sl]),
                "wv": WV,
                "wid": WID,
            }
        )

    res = run_bass_kernel_spmd(
        nc, in_maps, core_ids=list(range(N_CORES)), trace=_trace
    )
    _CACHE["last_result"] = res

    out_full = np.concatenate(
        [res.results[c]["out"] for c in range(N_CORES)], axis=2
    )  # [K, 16, N]
    outs = []
    offs = [0, 1, 4, 9, 16]
    for l in range(4):
        outs.append(
            np.ascontiguousarray(
                out_full[:, offs[l] : offs[l + 1], :].transpose(2, 1, 0)
            ).astype(np.float32)
        )
    return tuple(outs)
